# revision 12
# baseline (speedup 1.0000x reference)
# Trainium2 Bass kernel for DirectionalStockGNN (2-layer GATv2 + residual head).
#
# Sharding: edges are sorted by destination node on the host; each of the 8
# cores owns a contiguous range of N/8 destination nodes and all edges into
# them.  The segment softmax is then fully core-local (scores stay bounded,
# ~|e|<6, so no max-subtraction is needed).  The only collective is an
# AllGather of the layer-1 hidden state (bf16) between the two GAT layers.
#
# v2: all matmul operands are bf16 (single-pass PE, fast-weight-load) and the
# dense per-layer xl tables are gone: the kernel gathers raw x / h1 rows with
# a TRANSPOSED bf16 dma_gather (feature-major xgT) and applies Wl on the fly:
#   m[f',e]   = Wl.T @ xgT  +  [xr_win;We].T @ blobR(onehot;eaT)   (2 MM/group)
#   xe[e,f']  = xgT_blk.T @ Wl  (per-block stationary xgT)          (1 MM/blk)
#   pev[e]    = za_blk.T @ ones + zr_blk.T @ sgn                    (2 MM/blk)
#   pwin[w,:] += see_blk.T @ [xe | 1]   (dst-onehot*exp as lhsT)    (1 MM/blk)
# followed by a small per-window epilogue (divide, bias, ELU, transpose).
# Edges inside a window are sorted by src so gather indices fit int16
# relative to a per-call base row.

import math
import os

import numpy as np
import ml_dtypes

bft = ml_dtypes.bfloat16

D = 128
DE = 4
WIN = 124
NEG = 0.2
SPAN = 32768  # int16 index reach per gather call
CH = 8  # blocks per dma_gather call (1024 idxs, untransposed)


# ----------------------------------------------------------------------------
# host-side schedule + blob construction
# ----------------------------------------------------------------------------
def _wrap16(idx):
    """dma_gather index layout: [128, n/16] int16, wrap-16, replicated x8."""
    n = idx.shape[0]
    assert n % 16 == 0
    iw = np.zeros((16, n // 16), np.int16)
    iw[np.arange(n) % 16, np.arange(n) // 16] = idx
    return np.tile(iw, (8, 1))  # [128, n//16]


def build_host_data(x, edge_index, edge_attr, ncores):
    N = x.shape[0]
    src0 = np.asarray(edge_index[0], dtype=np.int64)
    dst0 = np.asarray(edge_index[1], dtype=np.int64)
    ea = np.asarray(edge_attr, dtype=np.float32)

    # self loops with mean edge_attr per dst (PyG fill_value='mean')
    sums = np.zeros((N, DE), np.float32)
    np.add.at(sums, dst0, ea)
    cnts = np.bincount(dst0, minlength=N).astype(np.float32)
    loop_attr = sums / np.maximum(cnts, 1.0)[:, None]

    src = np.concatenate([src0, np.arange(N, dtype=np.int64)])
    dst = np.concatenate([dst0, np.arange(N, dtype=np.int64)])
    eaa = np.concatenate([ea, loop_attr], axis=0)

    order = np.argsort(dst, kind="stable")
    src_s = src[order]
    dst_s = dst[order]
    ea_s = eaa[order]

    NPC = N // ncores
    NW = math.ceil(NPC / WIN)

    starts = np.minimum(np.arange(NW + 1) * WIN, NPC)
    bounds = np.empty((ncores, NW + 1), np.int64)
    for c in range(ncores):
        bounds[c] = np.searchsorted(dst_s, c * NPC + starts)

    # per (core, window): edges sorted by src; common block grid
    ecw = [[None] * NW for _ in range(ncores)]
    KW = np.ones(NW, np.int64)
    for c in range(ncores):
        for w in range(NW):
            lo, hi = bounds[c, w], bounds[c, w + 1]
            o = np.argsort(src_s[lo:hi], kind="stable")
            ecw[c][w] = (
                src_s[lo:hi][o],
                (dst_s[lo:hi][o] - (c * NPC + w * WIN)).astype(np.int64),
                ea_s[lo:hi][o],
            )
            KW[w] = max(KW[w], (hi - lo + 127) // 128)

    # gather call slots per window: (blk0, nblk, base) common across cores
    calls = []
    for w in range(NW):
        kw = int(KW[w])
        final = []
        stack = [(b0, min(CH, kw - b0)) for b0 in range(0, kw, CH)][::-1]
        while stack:
            b0, nb = stack.pop()
            lo_min = None
            hi_max = None
            for c in range(ncores):
                s = ecw[c][w][0]
                e0, e1 = b0 * 128, min((b0 + nb) * 128, len(s))
                if e1 <= e0:
                    continue
                smin, smax = int(s[e0]), int(s[e1 - 1])
                lo_min = smin if lo_min is None else min(lo_min, smin)
                hi_max = smax if hi_max is None else max(hi_max, smax)
            if lo_min is None:
                final.append((b0, nb, 0))
            elif hi_max - lo_min < SPAN:
                final.append((b0, nb, lo_min))
            else:
                assert nb > 1, "single block spans >= 32768 src range"
                h = nb // 2
                stack.append((b0 + h, nb - h))
                stack.append((b0, h))
        calls.append(final)

    koff = np.zeros(NW + 1, np.int64)
    for w in range(NW):
        koff[w + 1] = koff[w] + int(KW[w])
    KTOT = int(koff[NW])

    blobI = np.zeros((ncores, 128, 8 * KTOT), np.int16)
    blobS = np.zeros((ncores, 128, WIN * KTOT), bft)
    blobR = np.zeros((ncores, 128, 128 * KTOT), bft)
    for c in range(ncores):
        for w in range(NW):
            kw = int(KW[w])
            ko = int(koff[w])
            ew = kw * 128
            s, drel, eav = ecw[c][w]
            ne = len(s)
            # blobR: onehot(dst_rel) rows 0:124, eaT rows 124:128
            R = np.zeros((128, ew), np.float32)
            R[drel[:ne], np.arange(ne)] = 1.0
            R[124:128, :ne] = eav.T
            blobR[c, :, 128 * ko : 128 * ko + ew] = R.astype(bft)
            # blobS: edge-major onehot  S[e, WIN*(ko+b) + wd] = (drel[e]==wd)
            S = np.zeros((128, kw, WIN), np.float32)
            S[np.arange(ne) % 128, np.arange(ne) // 128, drel[:ne]] = 1.0
            blobS[c, :, WIN * ko : WIN * (ko + kw)] = S.reshape(128, kw * WIN).astype(
                bft
            )
            # blobI per call slot
            for b0, nb, base in calls[w]:
                e0, e1 = b0 * 128, min((b0 + nb) * 128, ne)
                rel = np.zeros(nb * 128, np.int64)
                if e1 > e0:
                    rel[: e1 - e0] = s[e0:e1] - base
                assert rel.min() >= 0 and rel.max() < SPAN
                blobI[c, :, 8 * (ko + b0) : 8 * (ko + b0 + nb)] = _wrap16(
                    rel.astype(np.int16)
                )

    sched = dict(
        N=N, NPC=NPC, NW=NW,
        KW=[int(k) for k in KW], koff=[int(v) for v in koff],
        calls=calls, ncores=ncores,
    )
    return sched, blobI, blobS, blobR


def build_consts(ins):
    f32 = np.float32
    x = np.ascontiguousarray(np.asarray(ins["x"], f32))
    consts = {}
    consts["xT_own_all"] = np.ascontiguousarray(x.T)  # [128, N] f32, split later
    consts["xb"] = np.ascontiguousarray(x.astype(bft))  # [N, 128] bf16 table
    for li in (1, 2):
        consts[f"wl{li}"] = np.asarray(ins[f"W{li}l"], f32).astype(bft)
        consts[f"wr{li}"] = np.asarray(ins[f"W{li}r"], f32).astype(bft)
        consts[f"we{li}"] = np.asarray(ins[f"W{li}e"], f32).astype(bft)  # [4,128]
        a = np.asarray(ins[f"att{li}"], f32)
        consts[f"attabs{li}"] = np.ascontiguousarray(np.abs(a)[:, None])
        consts[f"att02_{li}"] = np.ascontiguousarray(NEG * a[:, None])
        consts[f"sgn{li}"] = np.ascontiguousarray(
            ((1.0 - NEG) * np.sign(a))[:, None]
        ).astype(bft)
        b = np.asarray(ins[f"b{li}"], f32)
        consts[f"bb{li}"] = np.ascontiguousarray(np.tile(b[None, :], (D, 1)))
    consts["wfc"] = np.asarray(ins["Wfc"], f32).reshape(D, 1).astype(bft)
    consts["onec"] = np.ones((D, 1), bft)
    consts["zcol"] = np.zeros((D, 1), f32)
    consts["ident"] = np.eye(D, dtype=f32)
    consts["identb"] = np.eye(D, dtype=f32).astype(bft)
    return consts


# ----------------------------------------------------------------------------
# bass program
# ----------------------------------------------------------------------------
def build_program(sched, bfc_val):
    import concourse.bacc as bacc
    import concourse.mybir as mybir
    import concourse.tile as tile

    f32 = mybir.dt.float32
    bf = mybir.dt.bfloat16
    i16 = mybir.dt.int16
    Alu = mybir.AluOpType
    Act = mybir.ActivationFunctionType

    ncores = sched["ncores"]
    N, NPC, NW = sched["N"], sched["NPC"], sched["NW"]
    KW, koff, calls = sched["KW"], sched["koff"], sched["calls"]
    KTOT = koff[NW]
    KWMAX = max(KW)
    EWMAX = KWMAX * 128
    HT = NW * WIN

    nc = bacc.Bacc(
        "TRN2", target_bir_lowering=False, debug=False,
        enable_asserts=False, num_devices=ncores,
    )

    # ---- I/O ----
    t_xb = nc.dram_tensor("xb", [N, D], bf, kind="ExternalInput")
    t_xT_own = nc.dram_tensor("xT_own", [D, NPC], f32, kind="ExternalInput")
    t_xT_own_bf = nc.dram_tensor("xT_own_bf", [D, NPC], bf, kind="ExternalInput")
    t_blobI = nc.dram_tensor("blobI", [128, 8 * KTOT], i16, kind="ExternalInput")
    t_blobS = nc.dram_tensor("blobS", [128, WIN * KTOT], bf, kind="ExternalInput")
    t_blobR = nc.dram_tensor("blobR", [128, 128 * KTOT], bf, kind="ExternalInput")
    cshapes = dict(
        wl1=([D, D], bf), wr1=([D, D], bf), wl2=([D, D], bf), wr2=([D, D], bf),
        we1=([DE, D], bf), we2=([DE, D], bf),
        attabs1=([D, 1], f32), att02_1=([D, 1], f32), sgn1=([D, 1], bf),
        attabs2=([D, 1], f32), att02_2=([D, 1], f32), sgn2=([D, 1], bf),
        bb1=([D, D], f32), bb2=([D, D], f32), wfc=([D, 1], bf),
        onec=([D, 1], bf), zcol=([D, 1], f32),
        ident=([D, D], f32), identb=([D, D], bf),
    )
    t_c = {
        k: nc.dram_tensor(k, sh, dt, kind="ExternalInput")
        for k, (sh, dt) in cshapes.items()
    }
    t_y = nc.dram_tensor("y", [NPC, 1], f32, kind="ExternalOutput")

    # ---- DRAM internals ----
    t_h1own = nc.dram_tensor("h1own", [NPC, D], bf, kind="Internal")
    t_h1all = nc.dram_tensor(
        "h1all", [N, D], bf, kind="Internal",
        addr_space=("Shared" if ncores > 1 else "Local"),
    )

    with tile.TileContext(nc) as tc:
        with (
            tc.tile_pool(name="cpool", bufs=1) as cpool,
            tc.tile_pool(name="sp", bufs=3) as sp,
            tc.tile_pool(name="sp2", bufs=3) as sp2,
            tc.tile_pool(name="pm", bufs=1, space="PSUM") as pm_pool,
            tc.tile_pool(name="pxe", bufs=1, space="PSUM") as pxe_pool,
            tc.tile_pool(name="pt", bufs=2, space="PSUM") as pt_pool,
            tc.tile_pool(name="pwin", bufs=2, space="PSUM") as pwin_pool,
            tc.tile_pool(name="pe", bufs=1, space="PSUM") as pe_pool,
            tc.tile_pool(name="paux", bufs=1, space="PSUM") as paux_pool,
        ):
            # ---- load consts ----
            C = {}
            for k, (sh, dt) in cshapes.items():
                C[k] = cpool.tile(sh, dt, tag=f"c_{k}", name=f"c_{k}")
                nc.sync.dma_start(out=C[k][:], in_=t_c[k][:])

            lhsT_sb = cpool.tile([D, NW, D], bf, tag="lhsT_sb", name="lhsT_sb")
            nc.vector.memset(lhsT_sb[:, :, :], 0.0)
            hT_bf = cpool.tile([D, HT], bf, tag="hT_bf", name="hT_bf")
            y_sb = cpool.tile([1, HT], f32, tag="y_sb", name="y_sb")
            xres = cpool.tile([D, NPC], f32, tag="xres", name="xres")
            nc.sync.dma_start(out=xres[:, :], in_=t_xT_own[:, :])
            xbf = cpool.tile([D, NPC], bf, tag="xbf", name="xbf")
            nc.sync.dma_start(out=xbf[:, :], in_=t_xT_own_bf[:, :])

            def dense_xr(layer):
                wr = C[f"wr{layer}"]
                for w in range(NW):
                    wn = min(WIN, NPC - w * WIN)
                    if layer == 1:
                        lhs = xbf[:, w * WIN : w * WIN + wn]
                    else:
                        lhs = hT_bf[:, w * WIN : w * WIN + wn]
                    ps = paux_pool.tile([D, 129], f32, tag="paux", name="psx")
                    nc.tensor.matmul(
                        out=ps[:wn, :128], lhsT=lhs, rhs=wr[:, :],
                        start=True, stop=True,
                    )
                    nc.scalar.copy(out=lhsT_sb[:wn, w, 0:128], in_=ps[:wn, :128])
                    nc.sync.dma_start(
                        out=lhsT_sb[124:128, w, 0:128], in_=t_c[f"we{layer}"][:, :]
                    )

            def edge_pass(layer, t_tab, tab_rows):
                eplvl = int(os.environ.get("GNN_EPLVL", "5"))
                wlB = C[f"wl{layer}"]
                attabs = C[f"attabs{layer}"]
                att02 = C[f"att02_{layer}"]
                sgn = C[f"sgn{layer}"]
                for w in range(NW):
                    kw = KW[w]
                    ko = koff[w]
                    ew = kw * 128
                    wn = min(WIN, NPC - w * WIN)
                    it = sp2.tile([D, 8 * KWMAX], i16, tag="it", name="it")
                    nc.sync.dma_start(
                        out=it[:, : 8 * kw],
                        in_=t_blobI[:, 8 * ko : 8 * ko + 8 * kw],
                    )
                    st = sp2.tile([D, WIN * KWMAX], bf, tag="st", name="st")
                    nc.sync.dma_start(
                        out=st[:, : WIN * kw],
                        in_=t_blobS[:, WIN * ko : WIN * (ko + kw)],
                    )
                    rt = sp2.tile([D, EWMAX], bf, tag="rt", name="rt")
                    nc.sync.dma_start(
                        out=rt[:, :ew], in_=t_blobR[:, 128 * ko : 128 * ko + ew]
                    )
                    xg = sp2.tile([D, KWMAX, D], bf, tag="xg", name="xg")
                    for b0, nb, base in calls[w]:
                        hi = min(base + SPAN, tab_rows)
                        nc.gpsimd.dma_gather(
                            out_ap=xg[:, b0 : b0 + nb, :],
                            in_ap=t_tab[base:hi, :],
                            idxs_ap=it[:, 8 * b0 : 8 * (b0 + nb)],
                            num_idxs=nb * 128,
                            num_idxs_reg=nb * 128,
                            elem_size=D,
                        )
                    pwin = pwin_pool.tile([D, 132], f32, tag="pwin", name="pwin")
                    nblk = 0
                    if eplvl < 2:
                        continue
                    for t0 in range(0, kw, 4):
                        nb4 = min(4, kw - t0)
                        T = nb4 * 128
                        c0 = t0 * 128
                        # transpose gathered rows to feature-major xgT
                        pT = pt_pool.tile([D, 512], bf, tag="pt", name="pT")
                        for cb in range(nb4):
                            nc.tensor.matmul(
                                out=pT[:, cb * 128 : (cb + 1) * 128],
                                lhsT=xg[:, t0 + cb, :],
                                rhs=C["identb"][:, :],
                                is_transpose=True, start=True, stop=True,
                            )
                        xgT = sp.tile([D, 512], bf, tag="xgT", name="xgT")
                        nc.vector.tensor_scalar(
                            out=xgT[:, :T], in0=pT[:, :T],
                            scalar1=1.0, scalar2=None, op0=Alu.mult,
                        )
                        # m[f',e] = Wl.T@xgT + [xr;We].T@blobR  (feature-major)
                        pm = pm_pool.tile([D, 512], f32, tag="pm", name="pm")
                        nc.tensor.matmul(
                            out=pm[:, :T], lhsT=wlB[:, :],
                            rhs=xgT[:, :T], start=True, stop=False,
                        )
                        nc.tensor.matmul(
                            out=pm[:, :T], lhsT=lhsT_sb[:, w, :],
                            rhs=rt[:, c0 : c0 + T], start=False, stop=True,
                        )
                        if eplvl < 3:
                            continue
                        # xe[e,f'] = xgT_blk.T @ Wl  (edge-major xl rows)
                        pxe = pxe_pool.tile([D, 512], f32, tag="pxe", name="pxe")
                        for cb in range(nb4):
                            nc.tensor.matmul(
                                out=pxe[:, cb * 128 : (cb + 1) * 128],
                                lhsT=xgT[:, cb * 128 : (cb + 1) * 128],
                                rhs=wlB[:, :],
                                start=True, stop=True,
                            )
                        # za = 0.2*att*m ; zr = relu(|att|*m)  (bf16)
                        za = sp.tile([D, 512], bf, tag="za", name="za")
                        nc.scalar.activation(
                            out=za[:, :T], in_=pm[:, :T], func=Act.Copy,
                            scale=att02[:, :],
                        )
                        zr = sp.tile([D, 512], bf, tag="zr", name="zr")
                        nc.scalar.activation(
                            out=zr[:, :T], in_=pm[:, :T], func=Act.Relu,
                            scale=attabs[:, :], bias=C["zcol"][:, :],
                        )
                        if eplvl < 4:
                            continue
                        # scores: e = ones.T@za + 0.8*sgn.T@zr ; ee = exp(e)
                        pev = pe_pool.tile([D, 4], f32, tag="pe", name="pev")
                        for cb in range(nb4):
                            nc.tensor.matmul(
                                out=pev[:, cb : cb + 1],
                                lhsT=za[:, cb * 128 : (cb + 1) * 128],
                                rhs=C["onec"][:, :],
                                start=True, stop=False,
                            )
                            nc.tensor.matmul(
                                out=pev[:, cb : cb + 1],
                                lhsT=zr[:, cb * 128 : (cb + 1) * 128],
                                rhs=sgn[:, :],
                                start=False, stop=True,
                            )
                        ee = sp.tile([D, 4], f32, tag="ee", name="ee")
                        nc.scalar.activation(
                            out=ee[:, :nb4], in_=pev[:, :nb4],
                            func=Act.Exp, bias=C["zcol"][:, :],
                        )
                        # xe129 = [ee*xe | ee]  (bf16)
                        xe = sp.tile([D, 4, 129], bf, tag="xe", name="xe")
                        for cb in range(nb4):
                            nc.vector.tensor_scalar(
                                out=xe[:, cb, 0:128],
                                in0=pxe[:, cb * 128 : (cb + 1) * 128],
                                scalar1=ee[:, cb : cb + 1], scalar2=None,
                                op0=Alu.mult,
                            )
                            nc.scalar.copy(
                                out=xe[:, cb, 128:129], in_=ee[:, cb : cb + 1]
                            )
                        if eplvl < 5:
                            continue
                        for cb in range(nb4):
                            g = nblk + cb
                            nc.tensor.matmul(
                                out=pwin[0:WIN, 0:129],
                                lhsT=st[:, (t0 + cb) * WIN : (t0 + cb + 1) * WIN],
                                rhs=xe[:, cb, 0:129],
                                start=(g == 0), stop=(g == kw - 1),
                            )
                        nblk += nb4
                    # ---- window epilogue ----
                    if eplvl < 5:
                        continue
                    den = sp.tile([WIN, 1], f32, tag="den", name="den")
                    nc.vector.tensor_scalar(
                        out=den[:, :], in0=pwin[0:WIN, 128:129],
                        scalar1=1e-30, scalar2=None, op0=Alu.max,
                    )
                    rec = sp.tile([WIN, 1], f32, tag="rec", name="rec")
                    nc.vector.reciprocal(out=rec[:, :], in_=den[:, :])
                    hw_ = sp.tile([WIN, D], f32, tag="hw", name="hw_")
                    nc.vector.tensor_scalar(
                        out=hw_[:, :], in0=pwin[0:WIN, 0:128],
                        scalar1=rec[:, :], scalar2=None, op0=Alu.mult,
                    )
                    nc.vector.tensor_tensor(
                        out=hw_[:, :], in0=hw_[:, :], in1=C[f"bb{layer}"][0:WIN, :],
                        op=Alu.add,
                    )
                    # ELU: h - min(h,0) + exp(min(h,0)) - 1
                    tmin = sp.tile([WIN, D], f32, tag="tmin", name="tmin")
                    nc.vector.tensor_scalar(
                        out=tmin[:, :], in0=hw_[:, :], scalar1=0.0, scalar2=None,
                        op0=Alu.min,
                    )
                    uexp = sp.tile([WIN, D], f32, tag="uexp", name="uexp")
                    nc.scalar.activation(
                        out=uexp[:, :], in_=tmin[:, :], func=Act.Exp,
                        bias=C["zcol"][0:WIN, :],
                    )
                    nc.vector.tensor_tensor(
                        out=hw_[:, :], in0=hw_[:, :], in1=tmin[:, :], op=Alu.subtract
                    )
                    nc.vector.tensor_scalar(
                        out=uexp[:, :], in0=uexp[:, :], scalar1=-1.0, scalar2=None,
                        op0=Alu.add,
                    )
                    nc.vector.tensor_tensor(
                        out=hw_[:, :], in0=hw_[:, :], in1=uexp[:, :], op=Alu.add
                    )
                    hb = sp.tile([WIN, D], bf, tag="hb", name="hb")
                    nc.scalar.copy(out=hb[:, :], in_=hw_[:, :])
                    if layer == 1:
                        nc.sync.dma_start(
                            out=t_h1own[w * WIN : w * WIN + wn, :], in_=hb[:wn, :]
                        )
                    # transpose h window -> [128f, wn]
                    pt = paux_pool.tile([D, 129], f32, tag="paux", name="pt")
                    nc.tensor.matmul(
                        out=pt[:, 0:WIN], lhsT=hw_[:, :], rhs=C["ident"][0:WIN, 0:WIN],
                        is_transpose=True, start=True, stop=True,
                    )
                    if layer == 1:
                        nc.scalar.copy(
                            out=hT_bf[:, w * WIN : w * WIN + WIN], in_=pt[:, 0:WIN]
                        )
                    else:
                        h2t = sp.tile([D, WIN], f32, tag="h2t", name="h2t")
                        nc.scalar.copy(out=h2t[:, :], in_=pt[:, 0:WIN])
                        nc.vector.tensor_tensor(
                            out=h2t[:, :wn], in0=h2t[:, :wn],
                            in1=xres[:, w * WIN : w * WIN + wn], op=Alu.add,
                        )
                        hx = sp.tile([D, WIN], bf, tag="hx", name="hx")
                        nc.scalar.copy(out=hx[:, :wn], in_=h2t[:, :wn])
                        py = paux_pool.tile([D, 129], f32, tag="paux", name="py")
                        nc.tensor.matmul(
                            out=py[0:1, :wn], lhsT=C["wfc"][:, :], rhs=hx[:, :wn],
                            start=True, stop=True,
                        )
                        nc.scalar.activation(
                            out=y_sb[:, w * WIN : w * WIN + wn], in_=py[0:1, :wn],
                            func=Act.Copy, bias=float(bfc_val),
                        )

            # ---------------- phases (GNN_MAXPHASE truncates for bisect) ----
            maxphase = int(os.environ.get("GNN_MAXPHASE", "5"))

            if maxphase < 5:
                nc.vector.memset(y_sb[:, :], 0.0)
            dense_xr(1)
            if maxphase >= 2:
                edge_pass(1, t_xb, N)
            if maxphase >= 3:
                if ncores > 1:
                    nc.gpsimd.collective_compute(
                        "AllGather",
                        mybir.AluOpType.bypass,
                        replica_groups=[list(range(ncores))],
                        ins=[t_h1own[:, :]],
                        outs=[t_h1all[:, :]],
                    )
                else:
                    nc.sync.dma_start(out=t_h1all[:, :], in_=t_h1own[:, :])
            if maxphase >= 4:
                dense_xr(2)
            if maxphase >= 5:
                edge_pass(2, t_h1all, N)
            nc.sync.dma_start(out=t_y[:, 0], in_=y_sb[0:1, 0:NPC])

    nc.compile()
    return nc


# ----------------------------------------------------------------------------
# entry points
# ----------------------------------------------------------------------------
def prepare(inputs, ncores=8):
    x = np.asarray(inputs["x"], np.float32)
    sched, blobI, blobS, blobR = build_host_data(
        x, inputs["edge_index"], inputs["edge_attr"], ncores
    )
    consts = build_consts(inputs)
    bfc_val = float(np.asarray(inputs["bfc"]).reshape(-1)[0])
    nc = build_program(sched, bfc_val)
    NPC = sched["NPC"]
    xT_all = consts.pop("xT_own_all")
    in_maps = []
    for c in range(ncores):
        m = dict(consts)
        m["xT_own"] = np.ascontiguousarray(xT_all[:, c * NPC : (c + 1) * NPC])
        m["xT_own_bf"] = np.ascontiguousarray(m["xT_own"].astype(bft))
        m["blobI"] = np.ascontiguousarray(blobI[c])
        m["blobS"] = np.ascontiguousarray(blobS[c])
        m["blobR"] = np.ascontiguousarray(blobR[c])
        in_maps.append(m)
    return nc, in_maps, sched


def kernel(**inputs) -> np.ndarray:
    ncores = 8
    nc, in_maps, sched = prepare(inputs, ncores)
    from concourse.bass_utils import run_bass_kernel_spmd

    res = run_bass_kernel_spmd(nc, in_maps, core_ids=list(range(ncores)))
    y = np.concatenate([res.results[c]["y"] for c in range(ncores)], axis=0)
    return y.astype(np.float32)


# revision 15
# speedup vs baseline: 1.0052x; 1.0052x over previous
# Trainium2 Bass kernel for DirectionalStockGNN (2-layer GATv2 + residual head).
#
# Sharding: edges are sorted by destination node on the host; each of the 8
# cores owns a contiguous range of N/8 destination nodes and all edges into
# them.  The segment softmax is then fully core-local (scores stay bounded,
# ~|e|<6, so no max-subtraction is needed).  The only collective is an
# AllGather of the layer-1 hidden state (bf16) between the two GAT layers.
#
# v2: all matmul operands are bf16 (single-pass PE, fast-weight-load) and the
# dense per-layer xl tables are gone: the kernel gathers raw x / h1 rows with
# a TRANSPOSED bf16 dma_gather (feature-major xgT) and applies Wl on the fly:
#   m[f',e]   = Wl.T @ xgT  +  [xr_win;We].T @ blobR(onehot;eaT)   (2 MM/group)
#   xe[e,f']  = xgT_blk.T @ Wl  (per-block stationary xgT)          (1 MM/blk)
#   pev[e]    = za_blk.T @ ones + zr_blk.T @ sgn                    (2 MM/blk)
#   pwin[w,:] += see_blk.T @ [xe | 1]   (dst-onehot*exp as lhsT)    (1 MM/blk)
# followed by a small per-window epilogue (divide, bias, ELU, transpose).
# Edges inside a window are sorted by src so gather indices fit int16
# relative to a per-call base row.

import math
import os

import numpy as np
import ml_dtypes

bft = ml_dtypes.bfloat16

D = 128
DE = 4
WIN = 124
NEG = 0.2
SPAN = 32768  # int16 index reach per gather call
CH = 8  # blocks per dma_gather call (1024 idxs, untransposed)


# ----------------------------------------------------------------------------
# host-side schedule + blob construction
# ----------------------------------------------------------------------------
def _wrap16(idx):
    """dma_gather index layout: [128, n/16] int16, wrap-16, replicated x8."""
    n = idx.shape[0]
    assert n % 16 == 0
    iw = np.zeros((16, n // 16), np.int16)
    iw[np.arange(n) % 16, np.arange(n) // 16] = idx
    return np.tile(iw, (8, 1))  # [128, n//16]


def build_host_data(x, edge_index, edge_attr, ncores):
    N = x.shape[0]
    src0 = np.asarray(edge_index[0], dtype=np.int64)
    dst0 = np.asarray(edge_index[1], dtype=np.int64)
    ea = np.asarray(edge_attr, dtype=np.float32)

    # self loops with mean edge_attr per dst (PyG fill_value='mean')
    sums = np.zeros((N, DE), np.float32)
    np.add.at(sums, dst0, ea)
    cnts = np.bincount(dst0, minlength=N).astype(np.float32)
    loop_attr = sums / np.maximum(cnts, 1.0)[:, None]

    src = np.concatenate([src0, np.arange(N, dtype=np.int64)])
    dst = np.concatenate([dst0, np.arange(N, dtype=np.int64)])
    eaa = np.concatenate([ea, loop_attr], axis=0)

    order = np.argsort(dst, kind="stable")
    src_s = src[order]
    dst_s = dst[order]
    ea_s = eaa[order]

    NPC = N // ncores
    NW = math.ceil(NPC / WIN)

    starts = np.minimum(np.arange(NW + 1) * WIN, NPC)
    bounds = np.empty((ncores, NW + 1), np.int64)
    for c in range(ncores):
        bounds[c] = np.searchsorted(dst_s, c * NPC + starts)

    # per (core, window): edges sorted by src; common block grid
    ecw = [[None] * NW for _ in range(ncores)]
    KW = np.ones(NW, np.int64)
    for c in range(ncores):
        for w in range(NW):
            lo, hi = bounds[c, w], bounds[c, w + 1]
            o = np.argsort(src_s[lo:hi], kind="stable")
            ecw[c][w] = (
                src_s[lo:hi][o],
                (dst_s[lo:hi][o] - (c * NPC + w * WIN)).astype(np.int64),
                ea_s[lo:hi][o],
            )
            KW[w] = max(KW[w], (hi - lo + 127) // 128)

    # gather call slots per window: (blk0, nblk, base) common across cores
    calls = []
    for w in range(NW):
        kw = int(KW[w])
        final = []
        stack = [(b0, min(CH, kw - b0)) for b0 in range(0, kw, CH)][::-1]
        while stack:
            b0, nb = stack.pop()
            lo_min = None
            hi_max = None
            for c in range(ncores):
                s = ecw[c][w][0]
                e0, e1 = b0 * 128, min((b0 + nb) * 128, len(s))
                if e1 <= e0:
                    continue
                smin, smax = int(s[e0]), int(s[e1 - 1])
                lo_min = smin if lo_min is None else min(lo_min, smin)
                hi_max = smax if hi_max is None else max(hi_max, smax)
            if lo_min is None:
                final.append((b0, nb, 0))
            elif hi_max - lo_min < SPAN:
                final.append((b0, nb, lo_min))
            else:
                assert nb > 1, "single block spans >= 32768 src range"
                h = nb // 2
                stack.append((b0 + h, nb - h))
                stack.append((b0, h))
        calls.append(final)

    koff = np.zeros(NW + 1, np.int64)
    for w in range(NW):
        koff[w + 1] = koff[w] + int(KW[w])
    KTOT = int(koff[NW])

    blobI = np.zeros((ncores, 128, 8 * KTOT), np.int16)
    blobS = np.zeros((ncores, 128, WIN * KTOT), bft)
    blobR = np.zeros((ncores, 128, 128 * KTOT), bft)
    for c in range(ncores):
        for w in range(NW):
            kw = int(KW[w])
            ko = int(koff[w])
            ew = kw * 128
            s, drel, eav = ecw[c][w]
            ne = len(s)
            # blobR: onehot(dst_rel) rows 0:124, eaT rows 124:128
            R = np.zeros((128, ew), np.float32)
            R[drel[:ne], np.arange(ne)] = 1.0
            R[124:128, :ne] = eav.T
            blobR[c, :, 128 * ko : 128 * ko + ew] = R.astype(bft)
            # blobS: edge-major onehot  S[e, WIN*(ko+b) + wd] = (drel[e]==wd)
            S = np.zeros((128, kw, WIN), np.float32)
            S[np.arange(ne) % 128, np.arange(ne) // 128, drel[:ne]] = 1.0
            blobS[c, :, WIN * ko : WIN * (ko + kw)] = S.reshape(128, kw * WIN).astype(
                bft
            )
            # blobI per call slot
            for b0, nb, base in calls[w]:
                e0, e1 = b0 * 128, min((b0 + nb) * 128, ne)
                rel = np.zeros(nb * 128, np.int64)
                if e1 > e0:
                    rel[: e1 - e0] = s[e0:e1] - base
                assert rel.min() >= 0 and rel.max() < SPAN
                blobI[c, :, 8 * (ko + b0) : 8 * (ko + b0 + nb)] = _wrap16(
                    rel.astype(np.int16)
                )

    sched = dict(
        N=N, NPC=NPC, NW=NW,
        KW=[int(k) for k in KW], koff=[int(v) for v in koff],
        calls=calls, ncores=ncores,
    )
    return sched, blobI, blobS, blobR


def build_consts(ins):
    f32 = np.float32
    x = np.ascontiguousarray(np.asarray(ins["x"], f32))
    consts = {}
    consts["xT_own_all"] = np.ascontiguousarray(x.T)  # [128, N] f32, split later
    consts["xb"] = np.ascontiguousarray(x.astype(bft))  # [N, 128] bf16 table
    for li in (1, 2):
        consts[f"wl{li}"] = np.asarray(ins[f"W{li}l"], f32).astype(bft)
        consts[f"wr{li}"] = np.asarray(ins[f"W{li}r"], f32).astype(bft)
        consts[f"we{li}"] = np.asarray(ins[f"W{li}e"], f32).astype(bft)  # [4,128]
        a = np.asarray(ins[f"att{li}"], f32)
        consts[f"attabs{li}"] = np.ascontiguousarray(np.abs(a)[:, None])
        consts[f"att02_{li}"] = np.ascontiguousarray(NEG * a[:, None])
        consts[f"sgn{li}"] = np.ascontiguousarray(
            ((1.0 - NEG) * np.sign(a))[:, None]
        ).astype(bft)
        b = np.asarray(ins[f"b{li}"], f32)
        consts[f"bb{li}"] = np.ascontiguousarray(np.tile(b[None, :], (D, 1)))
    consts["wfc"] = np.asarray(ins["Wfc"], f32).reshape(D, 1).astype(bft)
    consts["onec"] = np.ones((D, 1), bft)
    consts["zcol"] = np.zeros((D, 1), f32)
    consts["ident"] = np.eye(D, dtype=f32)
    consts["identb"] = np.eye(D, dtype=f32).astype(bft)
    return consts


# ----------------------------------------------------------------------------
# bass program
# ----------------------------------------------------------------------------
def build_program(sched, bfc_val):
    import concourse.bacc as bacc
    import concourse.mybir as mybir
    import concourse.tile as tile

    f32 = mybir.dt.float32
    bf = mybir.dt.bfloat16
    i16 = mybir.dt.int16
    Alu = mybir.AluOpType
    Act = mybir.ActivationFunctionType

    ncores = sched["ncores"]
    N, NPC, NW = sched["N"], sched["NPC"], sched["NW"]
    KW, koff, calls = sched["KW"], sched["koff"], sched["calls"]
    KTOT = koff[NW]
    KWMAX = max(KW)
    EWMAX = KWMAX * 128
    HT = NW * WIN

    nc = bacc.Bacc(
        "TRN2", target_bir_lowering=False, debug=False,
        enable_asserts=False, num_devices=ncores,
    )

    # ---- I/O ----
    t_xb = nc.dram_tensor("xb", [N, D], bf, kind="ExternalInput")
    t_xT_own = nc.dram_tensor("xT_own", [D, NPC], f32, kind="ExternalInput")
    t_xT_own_bf = nc.dram_tensor("xT_own_bf", [D, NPC], bf, kind="ExternalInput")
    t_blobI = nc.dram_tensor("blobI", [128, 8 * KTOT], i16, kind="ExternalInput")
    t_blobS = nc.dram_tensor("blobS", [128, WIN * KTOT], bf, kind="ExternalInput")
    t_blobR = nc.dram_tensor("blobR", [128, 128 * KTOT], bf, kind="ExternalInput")
    cshapes = dict(
        wl1=([D, D], bf), wr1=([D, D], bf), wl2=([D, D], bf), wr2=([D, D], bf),
        we1=([DE, D], bf), we2=([DE, D], bf),
        attabs1=([D, 1], f32), att02_1=([D, 1], f32), sgn1=([D, 1], bf),
        attabs2=([D, 1], f32), att02_2=([D, 1], f32), sgn2=([D, 1], bf),
        bb1=([D, D], f32), bb2=([D, D], f32), wfc=([D, 1], bf),
        onec=([D, 1], bf), zcol=([D, 1], f32),
        ident=([D, D], f32), identb=([D, D], bf),
    )
    t_c = {
        k: nc.dram_tensor(k, sh, dt, kind="ExternalInput")
        for k, (sh, dt) in cshapes.items()
    }
    t_y = nc.dram_tensor("y", [NPC, 1], f32, kind="ExternalOutput")

    # ---- DRAM internals ----
    t_h1own = nc.dram_tensor("h1own", [NPC, D], bf, kind="Internal")
    t_h1all = nc.dram_tensor(
        "h1all", [N, D], bf, kind="Internal",
        addr_space=("Shared" if ncores > 1 else "Local"),
    )

    with tile.TileContext(nc) as tc:
        with (
            tc.tile_pool(name="cpool", bufs=1) as cpool,
            tc.tile_pool(name="sp", bufs=3) as sp,
            tc.tile_pool(name="sp2", bufs=2) as sp2,
            tc.tile_pool(name="pm", bufs=1, space="PSUM") as pm_pool,
            tc.tile_pool(name="pxe", bufs=1, space="PSUM") as pxe_pool,
            tc.tile_pool(name="pt", bufs=2, space="PSUM") as pt_pool,
            tc.tile_pool(name="pwin", bufs=2, space="PSUM") as pwin_pool,
            tc.tile_pool(name="pe", bufs=1, space="PSUM") as pe_pool,
            tc.tile_pool(name="paux", bufs=1, space="PSUM") as paux_pool,
        ):
            # ---- load consts ----
            C = {}
            for k, (sh, dt) in cshapes.items():
                C[k] = cpool.tile(sh, dt, tag=f"c_{k}", name=f"c_{k}")
                nc.sync.dma_start(out=C[k][:], in_=t_c[k][:])

            lhsT_sb = cpool.tile([D, NW, D], bf, tag="lhsT_sb", name="lhsT_sb")
            nc.vector.memset(lhsT_sb[:, :, :], 0.0)
            hT_bf = cpool.tile([D, HT], bf, tag="hT_bf", name="hT_bf")
            y_sb = cpool.tile([1, HT], f32, tag="y_sb", name="y_sb")
            xres = cpool.tile([D, NPC], f32, tag="xres", name="xres")
            nc.sync.dma_start(out=xres[:, :], in_=t_xT_own[:, :])
            xbf = cpool.tile([D, NPC], bf, tag="xbf", name="xbf")
            nc.sync.dma_start(out=xbf[:, :], in_=t_xT_own_bf[:, :])

            def dense_xr(layer):
                wr = C[f"wr{layer}"]
                for w in range(NW):
                    wn = min(WIN, NPC - w * WIN)
                    if layer == 1:
                        lhs = xbf[:, w * WIN : w * WIN + wn]
                    else:
                        lhs = hT_bf[:, w * WIN : w * WIN + wn]
                    ps = paux_pool.tile([D, 129], f32, tag="paux", name="psx")
                    nc.tensor.matmul(
                        out=ps[:wn, :128], lhsT=lhs, rhs=wr[:, :],
                        start=True, stop=True,
                    )
                    nc.scalar.copy(out=lhsT_sb[:wn, w, 0:128], in_=ps[:wn, :128])
                    nc.sync.dma_start(
                        out=lhsT_sb[124:128, w, 0:128], in_=t_c[f"we{layer}"][:, :]
                    )

            def edge_pass(layer, t_tab, tab_rows):
                eplvl = int(os.environ.get("GNN_EPLVL", "5"))
                wlB = C[f"wl{layer}"]
                attabs = C[f"attabs{layer}"]
                att02 = C[f"att02_{layer}"]
                sgn = C[f"sgn{layer}"]
                for w in range(NW):
                    kw = KW[w]
                    ko = koff[w]
                    ew = kw * 128
                    wn = min(WIN, NPC - w * WIN)
                    it = sp2.tile([D, 8 * KWMAX], i16, tag="it", name="it")
                    nc.sync.dma_start(
                        out=it[:, : 8 * kw],
                        in_=t_blobI[:, 8 * ko : 8 * ko + 8 * kw],
                    )
                    st = sp2.tile([D, WIN * KWMAX], bf, tag="st", name="st")
                    nc.sync.dma_start(
                        out=st[:, : WIN * kw],
                        in_=t_blobS[:, WIN * ko : WIN * (ko + kw)],
                    )
                    rt = sp2.tile([D, EWMAX], bf, tag="rt", name="rt")
                    nc.sync.dma_start(
                        out=rt[:, :ew], in_=t_blobR[:, 128 * ko : 128 * ko + ew]
                    )
                    xg = sp2.tile([D, KWMAX, D], bf, tag="xg", name="xg")
                    for b0, nb, base in calls[w]:
                        hi = min(base + SPAN, tab_rows)
                        nc.gpsimd.dma_gather(
                            out_ap=xg[:, b0 : b0 + nb, :],
                            in_ap=t_tab[base:hi, :],
                            idxs_ap=it[:, 8 * b0 : 8 * (b0 + nb)],
                            num_idxs=nb * 128,
                            num_idxs_reg=nb * 128,
                            elem_size=D,
                        )
                    pwin = pwin_pool.tile([D, 132], f32, tag="pwin", name="pwin")
                    nblk = 0
                    if eplvl < 2:
                        continue
                    for t0 in range(0, kw, 4):
                        nb4 = min(4, kw - t0)
                        T = nb4 * 128
                        c0 = t0 * 128
                        # transpose gathered rows to feature-major xgT
                        pT = pt_pool.tile([D, 512], bf, tag="pt", name="pT")
                        for cb in range(nb4):
                            nc.tensor.matmul(
                                out=pT[:, cb * 128 : (cb + 1) * 128],
                                lhsT=xg[:, t0 + cb, :],
                                rhs=C["identb"][:, :],
                                is_transpose=True, start=True, stop=True,
                            )
                        xgT = sp.tile([D, 512], bf, tag="xgT", name="xgT")
                        nc.vector.tensor_scalar(
                            out=xgT[:, :T], in0=pT[:, :T],
                            scalar1=1.0, scalar2=None, op0=Alu.mult,
                        )
                        # m[f',e] = Wl.T@xgT + [xr;We].T@blobR  (feature-major)
                        pm = pm_pool.tile([D, 512], f32, tag="pm", name="pm")
                        nc.tensor.matmul(
                            out=pm[:, :T], lhsT=wlB[:, :],
                            rhs=xgT[:, :T], start=True, stop=False,
                        )
                        nc.tensor.matmul(
                            out=pm[:, :T], lhsT=lhsT_sb[:, w, :],
                            rhs=rt[:, c0 : c0 + T], start=False, stop=True,
                        )
                        if eplvl < 3:
                            continue
                        # xe[e,f'] = xgT_blk.T @ Wl  (edge-major xl rows)
                        pxe = pxe_pool.tile([D, 512], f32, tag="pxe", name="pxe")
                        for cb in range(nb4):
                            nc.tensor.matmul(
                                out=pxe[:, cb * 128 : (cb + 1) * 128],
                                lhsT=xgT[:, cb * 128 : (cb + 1) * 128],
                                rhs=wlB[:, :],
                                start=True, stop=True,
                            )
                        # za = 0.2*att*m ; zr = relu(|att|*m)  (bf16)
                        za = sp.tile([D, 512], bf, tag="za", name="za")
                        nc.scalar.activation(
                            out=za[:, :T], in_=pm[:, :T], func=Act.Copy,
                            scale=att02[:, :],
                        )
                        zr = sp.tile([D, 512], bf, tag="zr", name="zr")
                        nc.scalar.activation(
                            out=zr[:, :T], in_=pm[:, :T], func=Act.Relu,
                            scale=attabs[:, :], bias=C["zcol"][:, :],
                        )
                        if eplvl < 4:
                            continue
                        # scores: e = ones.T@za + 0.8*sgn.T@zr ; ee = exp(e)
                        pev = pe_pool.tile([D, 4], f32, tag="pe", name="pev")
                        for cb in range(nb4):
                            nc.tensor.matmul(
                                out=pev[:, cb : cb + 1],
                                lhsT=za[:, cb * 128 : (cb + 1) * 128],
                                rhs=C["onec"][:, :],
                                start=True, stop=False,
                            )
                            nc.tensor.matmul(
                                out=pev[:, cb : cb + 1],
                                lhsT=zr[:, cb * 128 : (cb + 1) * 128],
                                rhs=sgn[:, :],
                                start=False, stop=True,
                            )
                        ee = sp.tile([D, 4], f32, tag="ee", name="ee")
                        nc.scalar.activation(
                            out=ee[:, :nb4], in_=pev[:, :nb4],
                            func=Act.Exp, bias=C["zcol"][:, :],
                        )
                        # xe129 = [ee*xe | ee]  (bf16)
                        xe = sp.tile([D, 4, 129], bf, tag="xe", name="xe")
                        for cb in range(nb4):
                            nc.vector.tensor_scalar(
                                out=xe[:, cb, 0:128],
                                in0=pxe[:, cb * 128 : (cb + 1) * 128],
                                scalar1=ee[:, cb : cb + 1], scalar2=None,
                                op0=Alu.mult,
                            )
                            nc.scalar.copy(
                                out=xe[:, cb, 128:129], in_=ee[:, cb : cb + 1]
                            )
                        if eplvl < 5:
                            continue
                        for cb in range(nb4):
                            g = nblk + cb
                            nc.tensor.matmul(
                                out=pwin[0:WIN, 0:129],
                                lhsT=st[:, (t0 + cb) * WIN : (t0 + cb + 1) * WIN],
                                rhs=xe[:, cb, 0:129],
                                start=(g == 0), stop=(g == kw - 1),
                            )
                        nblk += nb4
                    # ---- window epilogue ----
                    if eplvl < 5:
                        continue
                    den = sp.tile([WIN, 1], f32, tag="den", name="den")
                    nc.vector.tensor_scalar(
                        out=den[:, :], in0=pwin[0:WIN, 128:129],
                        scalar1=1e-30, scalar2=None, op0=Alu.max,
                    )
                    rec = sp.tile([WIN, 1], f32, tag="rec", name="rec")
                    nc.vector.reciprocal(out=rec[:, :], in_=den[:, :])
                    hw_ = sp.tile([WIN, D], f32, tag="hw", name="hw_")
                    nc.vector.tensor_scalar(
                        out=hw_[:, :], in0=pwin[0:WIN, 0:128],
                        scalar1=rec[:, :], scalar2=None, op0=Alu.mult,
                    )
                    nc.vector.tensor_tensor(
                        out=hw_[:, :], in0=hw_[:, :], in1=C[f"bb{layer}"][0:WIN, :],
                        op=Alu.add,
                    )
                    # ELU: h - min(h,0) + exp(min(h,0)) - 1
                    tmin = sp.tile([WIN, D], f32, tag="tmin", name="tmin")
                    nc.vector.tensor_scalar(
                        out=tmin[:, :], in0=hw_[:, :], scalar1=0.0, scalar2=None,
                        op0=Alu.min,
                    )
                    uexp = sp.tile([WIN, D], f32, tag="uexp", name="uexp")
                    nc.scalar.activation(
                        out=uexp[:, :], in_=tmin[:, :], func=Act.Exp,
                        bias=C["zcol"][0:WIN, :],
                    )
                    nc.vector.tensor_tensor(
                        out=hw_[:, :], in0=hw_[:, :], in1=tmin[:, :], op=Alu.subtract
                    )
                    nc.vector.tensor_scalar(
                        out=uexp[:, :], in0=uexp[:, :], scalar1=-1.0, scalar2=None,
                        op0=Alu.add,
                    )
                    nc.vector.tensor_tensor(
                        out=hw_[:, :], in0=hw_[:, :], in1=uexp[:, :], op=Alu.add
                    )
                    hb = sp.tile([WIN, D], bf, tag="hb", name="hb")
                    nc.scalar.copy(out=hb[:, :], in_=hw_[:, :])
                    if layer == 1:
                        nc.sync.dma_start(
                            out=t_h1own[w * WIN : w * WIN + wn, :], in_=hb[:wn, :]
                        )
                    # transpose h window -> [128f, wn]
                    pt = paux_pool.tile([D, 129], f32, tag="paux", name="pt")
                    nc.tensor.matmul(
                        out=pt[:, 0:WIN], lhsT=hw_[:, :], rhs=C["ident"][0:WIN, 0:WIN],
                        is_transpose=True, start=True, stop=True,
                    )
                    if layer == 1:
                        nc.scalar.copy(
                            out=hT_bf[:, w * WIN : w * WIN + WIN], in_=pt[:, 0:WIN]
                        )
                    else:
                        h2t = sp.tile([D, WIN], f32, tag="h2t", name="h2t")
                        nc.scalar.copy(out=h2t[:, :], in_=pt[:, 0:WIN])
                        nc.vector.tensor_tensor(
                            out=h2t[:, :wn], in0=h2t[:, :wn],
                            in1=xres[:, w * WIN : w * WIN + wn], op=Alu.add,
                        )
                        hx = sp.tile([D, WIN], bf, tag="hx", name="hx")
                        nc.scalar.copy(out=hx[:, :wn], in_=h2t[:, :wn])
                        py = paux_pool.tile([D, 129], f32, tag="paux", name="py")
                        nc.tensor.matmul(
                            out=py[0:1, :wn], lhsT=C["wfc"][:, :], rhs=hx[:, :wn],
                            start=True, stop=True,
                        )
                        nc.scalar.activation(
                            out=y_sb[:, w * WIN : w * WIN + wn], in_=py[0:1, :wn],
                            func=Act.Copy, bias=float(bfc_val),
                        )

            # ---------------- phases (GNN_MAXPHASE truncates for bisect) ----
            maxphase = int(os.environ.get("GNN_MAXPHASE", "5"))

            if maxphase < 5:
                nc.vector.memset(y_sb[:, :], 0.0)
            dense_xr(1)
            if maxphase >= 2:
                edge_pass(1, t_xb, N)
            if maxphase >= 3:
                if ncores > 1:
                    nc.gpsimd.collective_compute(
                        "AllGather",
                        mybir.AluOpType.bypass,
                        replica_groups=[list(range(ncores))],
                        ins=[t_h1own[:, :]],
                        outs=[t_h1all[:, :]],
                    )
                else:
                    nc.sync.dma_start(out=t_h1all[:, :], in_=t_h1own[:, :])
            if maxphase >= 4:
                dense_xr(2)
            if maxphase >= 5:
                edge_pass(2, t_h1all, N)
            nc.sync.dma_start(out=t_y[:, 0], in_=y_sb[0:1, 0:NPC])

    nc.compile()
    return nc


# ----------------------------------------------------------------------------
# entry points
# ----------------------------------------------------------------------------
def prepare(inputs, ncores=8):
    x = np.asarray(inputs["x"], np.float32)
    sched, blobI, blobS, blobR = build_host_data(
        x, inputs["edge_index"], inputs["edge_attr"], ncores
    )
    consts = build_consts(inputs)
    bfc_val = float(np.asarray(inputs["bfc"]).reshape(-1)[0])
    nc = build_program(sched, bfc_val)
    NPC = sched["NPC"]
    xT_all = consts.pop("xT_own_all")
    in_maps = []
    for c in range(ncores):
        m = dict(consts)
        m["xT_own"] = np.ascontiguousarray(xT_all[:, c * NPC : (c + 1) * NPC])
        m["xT_own_bf"] = np.ascontiguousarray(m["xT_own"].astype(bft))
        m["blobI"] = np.ascontiguousarray(blobI[c])
        m["blobS"] = np.ascontiguousarray(blobS[c])
        m["blobR"] = np.ascontiguousarray(blobR[c])
        in_maps.append(m)
    return nc, in_maps, sched


def kernel(**inputs) -> np.ndarray:
    ncores = 8
    nc, in_maps, sched = prepare(inputs, ncores)
    from concourse.bass_utils import run_bass_kernel_spmd

    res = run_bass_kernel_spmd(nc, in_maps, core_ids=list(range(ncores)))
    y = np.concatenate([res.results[c]["y"] for c in range(ncores)], axis=0)
    return y.astype(np.float32)
